# revision 1
# baseline (speedup 1.0000x reference)
"""Trainium2 Bass kernel for a GNN message-passing layer (8-core SPMD).

Math (reference):
    h   = [x[row], x[col], edge_attr] @ W1 + b1        # [E, 258] @ [258, 128]
    m   = relu(LN(h, g1, be1))
    m   = relu(m @ W2 + b2)
    aggr= segment_sum(m, row, N)
    u   = relu(LN([x, aggr] @ Wu + bu, gu, beu))
    out = x + u

Restructure: h@W1 = P[row] + Q[col] + edge_attr@W1c + b1 with P = x@W1a + b1,
Q = x@W1b, W1 = [W1a; W1b; W1c].  The big edge matmul becomes two node-table
gathers plus a rank-2 term.

Sharding: edges sorted by source row on the host, sharded by row range
(nloc nodes per core) so the segment-sum is core-local.  Edges are bucketed
into 128-node windows; a one-hot matrix per 128-edge tile scatters messages
into a per-window PSUM accumulator on the tensor engine.  Q is computed
shard-wise on-device and AllGathered.  Gathers use dma_gather (int16 idx,
<=32767) so the Q table is split in two halves and edges also bucketed by
col half ("phase").  One NEFF runs on all 8 cores: per-(window,phase) tile
counts are maxed across cores; pad edges gather row 0 and scatter to one-hot
column -1 (all-zero one-hot => no contribution).
"""

import math
import os
import numpy as np

D = 128
N_NODES = 50000
N_EDGES = 800000
N_CORES = 8
EPS = 1e-5
P = 128


class Cfg:
    def __init__(self, n_nodes, n_edges, n_cores):
        self.n_nodes = n_nodes
        self.n_edges = n_edges
        self.n_cores = n_cores
        assert n_nodes % n_cores == 0
        self.nloc = n_nodes // n_cores
        self.n_win = math.ceil(self.nloc / P)
        self.nloc_pad = self.n_win * P
        self.qsplit = (n_nodes + 1) // 2
        assert self.qsplit <= 32767 and n_nodes - self.qsplit <= 32767
        self.nt = None
        self.t_total = None
        self.e_pad = None
        self.triv = None


# ---------------------------------------------------------------- host prep

def _wrap16(idx, e_pad):
    """dma_gather index layout: idx j -> (partition j%16, col j//16),
    replicated 8x down the 128 partitions."""
    a = idx.reshape(e_pad // 16, 16).T.astype(np.int16)
    return np.ascontiguousarray(np.tile(a, (8, 1)))


def _ilv128(v):
    """Partition-interleave to match gather output: edge j -> [j%128, j//128]."""
    return np.ascontiguousarray(v.reshape(-1, P).T)


def preprocess(cfg, x, edge_index, edge_attr):
    rows = np.asarray(edge_index[0], dtype=np.int64)
    cols = np.asarray(edge_index[1], dtype=np.int64)
    attr = np.asarray(edge_attr, dtype=np.float32)

    order = np.argsort(rows, kind="stable")
    rs = rows[order]
    bounds = np.searchsorted(rs, np.arange(cfg.n_cores + 1) * cfg.nloc)

    per_core = []
    counts = np.zeros((cfg.n_cores, cfg.n_win, 2), dtype=np.int64)
    for k in range(cfg.n_cores):
        sel = order[bounds[k]:bounds[k + 1]]
        row_l = (rows[sel] - k * cfg.nloc).astype(np.int32)
        col = cols[sel].astype(np.int32)
        at = attr[sel]
        key = (row_l >> 7) * 2 + (col >= cfg.qsplit)
        o2 = np.argsort(key, kind="stable")
        row_l, col, at, key = row_l[o2], col[o2], at[o2], key[o2]
        cnt = np.bincount(key, minlength=cfg.n_win * 2).reshape(cfg.n_win, 2)
        counts[k] = cnt
        per_core.append((row_l, col, at, cnt))

    nt = np.ceil(counts.max(axis=0) / P).astype(np.int64)   # [n_win, 2]
    # every window must have at least one tile (update MLP runs per window)
    nt[:, 0] = np.maximum(nt[:, 0], 1)
    cfg.nt = nt
    cfg.t_total = int(nt.sum())
    cfg.e_pad = cfg.t_total * P

    core_arrays = []
    for k in range(cfg.n_cores):
        row_l, col, at, cnt = per_core[k]
        qidx = np.zeros(cfg.e_pad, dtype=np.int32)
        pidx = np.zeros(cfg.e_pad, dtype=np.int32)
        sidx = np.full(cfg.e_pad, -1.0, dtype=np.float32)
        a0 = np.zeros(cfg.e_pad, dtype=np.float32)
        a1 = np.zeros(cfg.e_pad, dtype=np.float32)
        src = 0
        dst = 0
        for w in range(cfg.n_win):
            for ph in range(2):
                c = int(cnt[w, ph])
                seg = slice(src, src + c)
                d = slice(dst, dst + c)
                qidx[d] = col[seg] - ph * cfg.qsplit
                pidx[d] = row_l[seg]
                sidx[d] = row_l[seg] - w * P
                a0[d] = at[seg, 0]
                a1[d] = at[seg, 1]
                src += c
                dst += int(nt[w, ph]) * P
        core_arrays.append(dict(
            qidx=_wrap16(qidx, cfg.e_pad),
            pidx=_wrap16(pidx, cfg.e_pad),
            sidx=_ilv128(sidx),
            a0=_ilv128(a0),
            a1=_ilv128(a1),
        ))
    return core_arrays


# ---------------------------------------------------------------- device IR

def build(nc, tc, cfg, io):
    import concourse.bass as bass
    from concourse import mybir
    from concourse.masks import make_identity
    from contextlib import ExitStack

    f32 = mybir.dt.float32
    i16 = mybir.dt.int16
    AF = mybir.ActivationFunctionType
    OP = mybir.AluOpType
    triv = cfg.triv

    ctx = ExitStack()
    sing = ctx.enter_context(tc.tile_pool(name="sing", bufs=1))
    work = ctx.enter_context(tc.tile_pool(name="work", bufs=3))
    gat = ctx.enter_context(tc.tile_pool(name="gat", bufs=2))
    stat = ctx.enter_context(tc.tile_pool(name="stat", bufs=2))
    psum = ctx.enter_context(tc.tile_pool(name="psum", bufs=1, space="PSUM"))
    dram = ctx.enter_context(tc.tile_pool(name="dram", bufs=1, space="DRAM"))

    # ---------------- resident weights / constants
    def load_w(name):
        t = sing.tile([P, D], f32, name=f"{name}_sb")
        nc.sync.dma_start(out=t[:], in_=io[name][:])
        return t

    w1a, w1b, w2, wux, wua = (load_w(n) for n in
                              ("W1a", "W1b", "W2", "Wu_x", "Wu_a"))

    def bcast_row(name):
        t = sing.tile([P, D], f32, name=f"{name}_b")
        src = io[name]
        ap = bass.AP(tensor=src.tensor, offset=src.offset,
                     ap=[[0, P]] + list(src.ap))
        nc.sync.dma_start(out=t[:], in_=ap)
        return t

    c0_b = bcast_row("c0")
    c1_b = bcast_row("c1")
    iota_b = bcast_row("iota")
    b1_b = None if triv["b1"] else bcast_row("b1")
    b2_b = None if triv["b2"] else bcast_row("b2")
    bu_b = None if triv["bu"] else bcast_row("bu")
    g1_b = None if triv["g1"] else bcast_row("g1")
    be1_b = None if triv["be1"] else bcast_row("be1")
    gu_b = None if triv["gu"] else bcast_row("gu")
    beu_b = None if triv["beu"] else bcast_row("beu")

    ident = sing.tile([P, P], f32, name="ident")
    make_identity(nc, ident[:])

    eps_t = sing.tile([P, 1], f32, name="eps_t")
    nc.vector.memset(eps_t[:], EPS)

    xT = sing.tile([P, cfg.nloc_pad], f32, name="xT_sb")
    nc.sync.dma_start(out=xT[:], in_=io["xT_local"][:])

    cols16 = cfg.e_pad // 16
    qidx = sing.tile([P, cols16], i16, name="qidx_sb")
    nc.sync.dma_start(out=qidx[:], in_=io["qidx"][:])
    pidx = sing.tile([P, cols16], i16, name="pidx_sb")
    nc.sync.dma_start(out=pidx[:], in_=io["pidx"][:])
    sidx = sing.tile([P, cfg.t_total], f32, name="sidx_sb")
    nc.sync.dma_start(out=sidx[:], in_=io["sidx"][:])
    a0 = sing.tile([P, cfg.t_total], f32, name="a0_sb")
    nc.sync.dma_start(out=a0[:], in_=io["a0"][:])
    a1 = sing.tile([P, cfg.t_total], f32, name="a1_sb")
    nc.sync.dma_start(out=a1[:], in_=io["a1"][:])

    # ---------------- DRAM intermediates
    q_shard = dram.tile([cfg.nloc, D], f32, name="q_shard")
    q_full = dram.tile([cfg.n_nodes, D], f32, name="q_full",
                       addr_space="Shared")
    p_dram = dram.tile([cfg.nloc_pad, D], f32, name="p_dram")

    # ---------------- phase A: node projections
    for t in range(cfg.n_win):
        n0 = t * P
        qp = psum.tile([P, D], f32, name="qp", tag="mmB", bufs=3)
        nc.tensor.matmul(qp[:], lhsT=xT[:, n0:n0 + P], rhs=w1b[:],
                         start=True, stop=True)
        qs = work.tile([P, D], f32, name="qs", tag="qs")
        nc.scalar.copy(out=qs[:], in_=qp[:])
        hi = min(n0 + P, cfg.nloc)
        if hi > n0:
            nc.sync.dma_start(out=q_shard[n0:hi, :], in_=qs[:hi - n0, :])

        pp = psum.tile([P, D], f32, name="pp", tag="mmB", bufs=3)
        nc.tensor.matmul(pp[:], lhsT=xT[:, n0:n0 + P], rhs=w1a[:],
                         start=True, stop=True)
        ps = work.tile([P, D], f32, name="ps", tag="ps")
        if b1_b is None:
            nc.scalar.copy(out=ps[:], in_=pp[:])
        else:
            nc.vector.tensor_add(out=ps[:], in0=pp[:], in1=b1_b[:])
        nc.sync.dma_start(out=p_dram[n0:n0 + P, :], in_=ps[:])

    nc.gpsimd.collective_compute(
        "AllGather",
        mybir.AluOpType.bypass,
        ins=[q_shard[:].opt()],
        outs=[q_full[:].opt()],
        replica_groups=[list(range(cfg.n_cores))],
    )

    # ---------------- phase B: edge pipeline
    out_dram = io["out"]
    q_lo = q_full[0:cfg.qsplit, :]
    q_hi = q_full[cfg.qsplit:cfg.n_nodes, :]
    nt = cfg.nt
    max_ntw = int((nt[:, 0] + nt[:, 1]).max())

    jt_base = 0
    for w in range(cfg.n_win):
        ntw = int(nt[w, 0] + nt[w, 1])
        assert ntw > 0
        e0 = jt_base * P

        pg = gat.tile([P, max_ntw, D], f32, name="pg", tag="pg")
        nc.gpsimd.dma_gather(
            out_ap=pg[:, 0:ntw, :], in_ap=p_dram[:],
            idxs_ap=pidx[:, e0 // 16:(e0 + ntw * P) // 16],
            num_idxs=ntw * P, num_idxs_reg=ntw * P, elem_size=D,
            single_packet=False,
        )
        qg = gat.tile([P, max_ntw, D], f32, name="qg", tag="qg")
        boff = 0
        for ph in range(2):
            ntb = int(nt[w, ph])
            if ntb == 0:
                continue
            g0 = e0 + boff * P
            nc.gpsimd.dma_gather(
                out_ap=qg[:, boff:boff + ntb, :],
                in_ap=(q_lo if ph == 0 else q_hi),
                idxs_ap=qidx[:, g0 // 16:(g0 + ntb * P) // 16],
                num_idxs=ntb * P, num_idxs_reg=ntb * P, elem_size=D,
                single_packet=False,
            )
            boff += ntb

        zg = gat.tile([P, max_ntw, D], f32, name="zg", tag="zg")
        mv = stat.tile([P, max_ntw, 2], f32, name="mv", tag="mv")
        for i in range(ntw):
            jt = jt_base + i
            zt = zg[:, i, :]
            nc.vector.scalar_tensor_tensor(
                out=zt, in0=c0_b[:], scalar=a0[:, jt:jt + 1], in1=qg[:, i, :],
                op0=OP.mult, op1=OP.add)
            nc.vector.scalar_tensor_tensor(
                out=zt, in0=c1_b[:], scalar=a1[:, jt:jt + 1], in1=zt,
                op0=OP.mult, op1=OP.add)
            nc.vector.tensor_add(out=zt, in0=zt, in1=pg[:, i, :])
            st6 = stat.tile([P, 6], f32, name="st6", tag="st6")
            nc.vector.bn_stats(out=st6[:], in_=zt)
            nc.vector.bn_aggr(out=mv[:, i, :], in_=st6[:])

        rstd = stat.tile([P, max_ntw], f32, name="rstd", tag="rstd")
        nmu = stat.tile([P, max_ntw], f32, name="nmu", tag="nmu")
        nc.scalar.activation(out=rstd[:, 0:ntw], in_=mv[:, 0:ntw, 1],
                             func=AF.Sqrt, bias=eps_t[:], scale=1.0)
        nc.vector.reciprocal(out=rstd[:, 0:ntw], in_=rstd[:, 0:ntw])
        nc.vector.scalar_tensor_tensor(
            out=nmu[:, 0:ntw], in0=mv[:, 0:ntw, 0], scalar=-1.0,
            in1=rstd[:, 0:ntw], op0=OP.mult, op1=OP.mult)

        agg = psum.tile([P, P], f32, name="agg", tag="agg", bufs=2)
        for i in range(ntw):
            jt = jt_base + i
            zt = zg[:, i, :]
            m1 = work.tile([P, D], f32, name="m1", tag="m1")
            if triv["g1"] and triv["be1"]:
                nc.scalar.activation(out=m1[:], in_=zt, func=AF.Relu,
                                     scale=rstd[:, i:i + 1],
                                     bias=nmu[:, i:i + 1])
            else:
                nc.scalar.activation(out=m1[:], in_=zt, func=AF.Identity,
                                     scale=rstd[:, i:i + 1],
                                     bias=nmu[:, i:i + 1])
                nc.vector.tensor_mul(out=m1[:], in0=m1[:], in1=g1_b[:])
                nc.vector.tensor_add(out=m1[:], in0=m1[:], in1=be1_b[:])
                nc.vector.tensor_scalar_max(out=m1[:], in0=m1[:], scalar1=0.0)

            trp = psum.tile([P, P], f32, name="trp", tag="mmB", bufs=3)
            nc.tensor.transpose(out=trp[:], in_=m1[:], identity=ident[:])
            m1t = work.tile([P, D], f32, name="m1t", tag="m1t")
            nc.vector.tensor_copy(out=m1t[:], in_=trp[:])

            m2p = psum.tile([P, D], f32, name="m2p", tag="mmC", bufs=2)
            nc.tensor.matmul(m2p[:], lhsT=m1t[:], rhs=w2[:],
                             start=True, stop=True)
            m2 = work.tile([P, D], f32, name="m2", tag="m2")
            if b2_b is not None:
                nc.vector.tensor_add(out=m2p[:], in0=m2p[:], in1=b2_b[:])
            nc.scalar.activation(out=m2[:], in_=m2p[:], func=AF.Relu)

            s1h = work.tile([P, P], f32, name="s1h", tag="s1h")
            nc.vector.tensor_scalar(
                out=s1h[:], in0=iota_b[:], scalar1=sidx[:, jt:jt + 1],
                scalar2=None, op0=OP.is_equal)
            nc.tensor.matmul(agg[:], lhsT=m2[:], rhs=s1h[:],
                             start=(i == 0), stop=(i == ntw - 1))

        # ---- update MLP for window w
        aggs = work.tile([P, P], f32, name="aggs", tag="aggs")
        nc.scalar.copy(out=aggs[:], in_=agg[:])

        up = psum.tile([P, D], f32, name="up", tag="mmB", bufs=3)
        nc.tensor.matmul(up[:], lhsT=aggs[:], rhs=wua[:],
                         start=True, stop=False)
        nc.tensor.matmul(up[:], lhsT=xT[:, w * P:(w + 1) * P], rhs=wux[:],
                         start=False, stop=True)
        if bu_b is not None:
            nc.vector.tensor_add(out=up[:], in0=up[:], in1=bu_b[:])

        st6u = stat.tile([P, 6], f32, name="st6u", tag="st6")
        mvu = stat.tile([P, 2], f32, name="mvu", tag="mvu")
        nc.vector.bn_stats(out=st6u[:], in_=up[:])
        nc.vector.bn_aggr(out=mvu[:], in_=st6u[:])
        rsu = stat.tile([P, 1], f32, name="rsu", tag="rsu")
        nmuu = stat.tile([P, 1], f32, name="nmuu", tag="nmuu")
        nc.scalar.activation(out=rsu[:], in_=mvu[:, 1:2], func=AF.Sqrt,
                             bias=eps_t[:], scale=1.0)
        nc.vector.reciprocal(out=rsu[:], in_=rsu[:])
        nc.vector.scalar_tensor_tensor(
            out=nmuu[:], in0=mvu[:, 0:1], scalar=-1.0, in1=rsu[:],
            op0=OP.mult, op1=OP.mult)

        u_sb = work.tile([P, D], f32, name="u_sb", tag="u_sb")
        if triv["gu"] and triv["beu"]:
            nc.scalar.activation(out=u_sb[:], in_=up[:], func=AF.Relu,
                                 scale=rsu[:], bias=nmuu[:])
        else:
            nc.scalar.activation(out=u_sb[:], in_=up[:], func=AF.Identity,
                                 scale=rsu[:], bias=nmuu[:])
            nc.vector.tensor_mul(out=u_sb[:], in0=u_sb[:], in1=gu_b[:])
            nc.vector.tensor_add(out=u_sb[:], in0=u_sb[:], in1=beu_b[:])
            nc.vector.tensor_scalar_max(out=u_sb[:], in0=u_sb[:], scalar1=0.0)

        xw = work.tile([P, D], f32, name="xw", tag="xw")
        nc.sync.dma_start(out=xw[:], in_=io["x_local"][w * P:(w + 1) * P, :])
        o_sb = work.tile([P, D], f32, name="o_sb", tag="o_sb")
        nc.vector.tensor_add(out=o_sb[:], in0=u_sb[:], in1=xw[:])
        nc.sync.dma_start(out=out_dram[w * P:(w + 1) * P, :], in_=o_sb[:])

        jt_base += ntw

    ctx.close()


def make_program(cfg):
    import concourse.bacc as bacc
    import concourse.tile as tile
    from concourse import mybir

    f32 = mybir.dt.float32
    i16 = mybir.dt.int16

    nc = bacc.Bacc("TRN2", target_bir_lowering=False, debug=False,
                   num_devices=cfg.n_cores)
    io = {}

    def din(name, shape, dtype=f32):
        io[name] = nc.dram_tensor(name, list(shape), dtype,
                                  kind="ExternalInput").ap()

    din("xT_local", [P, cfg.nloc_pad])
    din("x_local", [cfg.nloc_pad, D])
    din("qidx", [P, cfg.e_pad // 16], i16)
    din("pidx", [P, cfg.e_pad // 16], i16)
    din("sidx", [P, cfg.t_total])
    din("a0", [P, cfg.t_total])
    din("a1", [P, cfg.t_total])
    for nm in ("W1a", "W1b", "W2", "Wu_x", "Wu_a"):
        din(nm, [P, D])
    for nm in ("c0", "c1", "iota", "b1", "b2", "bu", "g1", "be1", "gu", "beu"):
        din(nm, [D])
    io["out"] = nc.dram_tensor("out", [cfg.nloc_pad, D], f32,
                               kind="ExternalOutput").ap()

    with tile.TileContext(nc) as tc:
        build(nc, tc, cfg, io)
    nc.compile()
    return nc


# ---------------------------------------------------------------- entry

def _is_const(v, val):
    return bool(np.allclose(np.asarray(v), val))


def kernel(x, edge_index, edge_attr, W1, b1, g1, be1, W2, b2, Wu, bu, gu, beu,
           cfg=None, run=True):
    x = np.ascontiguousarray(np.asarray(x, dtype=np.float32))
    edge_index = np.asarray(edge_index)
    edge_attr = np.asarray(edge_attr, dtype=np.float32)
    W1 = np.asarray(W1, dtype=np.float32)
    W2 = np.ascontiguousarray(np.asarray(W2, dtype=np.float32))
    Wu = np.asarray(Wu, dtype=np.float32)

    if cfg is None:
        cfg = Cfg(N_NODES, N_EDGES, N_CORES)
    cfg.triv = dict(
        b1=_is_const(b1, 0), b2=_is_const(b2, 0), bu=_is_const(bu, 0),
        g1=_is_const(g1, 1), be1=_is_const(be1, 0),
        gu=_is_const(gu, 1), beu=_is_const(beu, 0),
    )

    core_arrays = preprocess(cfg, x, edge_index, edge_attr)

    weights = dict(
        W1a=np.ascontiguousarray(W1[0:D]),
        W1b=np.ascontiguousarray(W1[D:2 * D]),
        W2=W2,
        Wu_x=np.ascontiguousarray(Wu[0:D]),
        Wu_a=np.ascontiguousarray(Wu[D:2 * D]),
        c0=np.ascontiguousarray(W1[2 * D]),
        c1=np.ascontiguousarray(W1[2 * D + 1]),
        iota=np.arange(P, dtype=np.float32),
        b1=np.ascontiguousarray(np.asarray(b1, np.float32)),
        b2=np.ascontiguousarray(np.asarray(b2, np.float32)),
        bu=np.ascontiguousarray(np.asarray(bu, np.float32)),
        g1=np.ascontiguousarray(np.asarray(g1, np.float32)),
        be1=np.ascontiguousarray(np.asarray(be1, np.float32)),
        gu=np.ascontiguousarray(np.asarray(gu, np.float32)),
        beu=np.ascontiguousarray(np.asarray(beu, np.float32)),
    )

    nc = make_program(cfg)

    in_maps = []
    for k in range(cfg.n_cores):
        xl = np.zeros((cfg.nloc_pad, D), dtype=np.float32)
        xl[:cfg.nloc] = x[k * cfg.nloc:(k + 1) * cfg.nloc]
        m = dict(core_arrays[k])
        m["x_local"] = xl
        m["xT_local"] = np.ascontiguousarray(xl.T)
        m.update(weights)
        in_maps.append(m)

    if not run:
        return nc, in_maps, cfg

    from concourse import bass_utils
    res = bass_utils.run_bass_kernel_spmd(
        nc, in_maps, core_ids=list(range(cfg.n_cores)),
        trace=bool(int(os.environ.get("KERNEL_TRACE", "0"))),
    )
    kernel.last_results = res
    outs = [r["out"][:cfg.nloc] for r in res.results]
    return np.concatenate(outs, axis=0)


kernel.last_results = None

